# revision 8
# baseline (speedup 1.0000x reference)
"""Trainium2 Bass kernel for masked attention-pooling (DmasifAttentionModule).

Reference computation (per sample b):
    proj   = x @ W.T + b                  # [N, D]
    scores = proj @ v                     # [N]
    scores = where(mask, scores, -1e9)
    w      = softmax(scores)              # [N]
    out    = w @ x                        # [D]

Algebraic collapse used here (exact up to fp reassociation):
    scores = x @ (W.T @ v) + (b . v)
and softmax is shift-invariant, so the (b . v) constant drops out entirely.
With u = v @ W (a 512-vector computed on host), the device work is:
    s[n]  = sum_d (x[n,d] + mbias[n]) * u[d]   # = x@u + mbias[n]*S_u
    e     = exp(s - C)                         # C folded into ACT bias tile
    Z     = sum_n e[n]                         # free accum_out of the exp op
    out   = (sum_n e[n] * x[n,:]) / Z

The mask rides the AFFINE_MUL_REDUCE per-partition bias slot: mbias[n] = 0
for valid rows (bit-exact scores) and MASKED_INIT/S_u for masked rows (their
accumulated score lands at ~MASKED_INIT, so exp underflows to exactly 0).
S_u = sum(u) is computed on host; if it is degenerately small we fall back to
an explicit DVE mask-add.

Per-core layout (8 cores, 2 samples each, data-parallel over batch):
    - x shard [2, 4096, 512] f32 streamed as 16 x 1MiB tiles [128, 4, 512]
      (partition = n%128, free = (n//128 % 4, d)); tiles stay resident in SBUF
      so x is read from HBM exactly once.
    - scores: DVE AFFINE_MUL_REDUCE (fused mul + free-dim accum-reduce,
      ~1 elem/cycle; the native tensor_tensor_reduce opcode hard-crashes this
      runtime and scalar_tensor_tensor runs at half rate).
    - exp + Z-partials: one ScalarE activation per sample ([128, 32],
      bias = -C tile, accum_out = per-partition sums of e).
    - pooling + Z: TensorE matvec accumulation into PSUM
      (lhsT = e column [128,1], rhs = x chunk [128,512], plus ones^T @ zc).
This is HBM-bandwidth bound: 16 MiB/core read once (~47 us at ~358 GB/s).
"""

import os
import sys

import numpy as np

for _p in ("/opt/trn_rl_repo", "/root/.axon_site/_ro/trn_rl_repo"):
    if os.path.isdir(_p) and _p not in sys.path:
        sys.path.append(_p)

import concourse.bacc as bacc
import concourse.tile as tile
from concourse import mybir
from concourse.bass_utils import run_bass_kernel_spmd

B, N, D = 16, 4096, 512
N_CORES = 8
SPB = B // N_CORES          # samples per core
TILES = 8                   # 1MiB x-tiles per sample
COLS = N // 128             # 32 score columns of 128 n's per sample
CPT = COLS // TILES         # score columns per tile (4)
C_SHIFT = 24.0              # constant exp-range shift (softmax-invariant)
MASKED_INIT = -3.0e8        # masked scores -> exp underflows to exactly 0

_F32 = mybir.dt.float32
_CACHE = {}


def _build_program(unroll=1, mask_in_amr=True):
    """unroll>1 repeats the whole computation in one NEFF (timing only).

    mask_in_amr=True folds the mask into AFFINE_MUL_REDUCE's bias slot
    (mb input = 0 / MASKED_INIT/S_u); False applies mb additively with a
    DVE tensor_add before the exp (mb input = -C / MASKED_INIT)."""
    nc = bacc.Bacc("TRN2", target_bir_lowering=False, debug=False)
    x = nc.dram_tensor("x", [SPB, N, D], _F32, kind="ExternalInput").ap()
    mb = nc.dram_tensor("mb", [SPB, 128, COLS], _F32, kind="ExternalInput").ap()
    u = nc.dram_tensor("u", [128, D], _F32, kind="ExternalInput").ap()
    out = nc.dram_tensor("out", [SPB, D], _F32, kind="ExternalOutput").ap()

    # [s, i, p, c, d]: n = i*512 + c*128 + p
    x5 = x.rearrange("s (i c p) d -> s i p c d", i=TILES, c=CPT, p=128)

    with tile.TileContext(nc) as tc:
        with (
            tc.tile_pool(name="xp", bufs=SPB * TILES) as xp,
            tc.tile_pool(name="singles", bufs=1) as sg,
            tc.tile_pool(name="scratch", bufs=4) as scr,
            tc.tile_pool(name="smalls", bufs=2) as sm,
            tc.tile_pool(name="ps", bufs=2, space="PSUM") as psp,
        ):
            ones_sb = sg.tile([128, 1], _F32)
            nc.vector.memset(ones_sb[:], 1.0)
            shift_sb = sg.tile([128, 1], _F32)
            nc.vector.memset(shift_sb[:], -C_SHIFT)
            warm = sg.tile([128, 1], _F32)
            # Pull the exp table-set load (~2.7us) to t=0, under the DMAs.
            nc.scalar.activation(warm[:], ones_sb[:],
                                 mybir.ActivationFunctionType.Exp)

            u_sb = sg.tile([128, D], _F32)
            nc.sync.dma_start(out=u_sb[:], in_=u[:])
            mb_sb = sg.tile([128, SPB, COLS], _F32)
            nc.sync.dma_start(out=mb_sb[:], in_=mb.rearrange("s p c -> p s c"))

            s_sb = sg.tile([128, SPB, COLS], _F32)
            e_sb = sg.tile([128, SPB, COLS], _F32)
            zc_sb = sg.tile([128, SPB], _F32)

            for _it in range(unroll):
                _emit_iteration(nc, xp, scr, sm, psp, x5, out, u_sb, mb_sb,
                                ones_sb, shift_sb, s_sb, e_sb, zc_sb,
                                mask_in_amr)

    nc.compile()
    return nc


def _emit_iteration(nc, xp, scr, sm, psp, x5, out, u_sb, mb_sb, ones_sb,
                    shift_sb, s_sb, e_sb, zc_sb, mask_in_amr):
    x_tiles = {}
    for s in range(SPB):
        for i in range(TILES):
            t = xp.tile([128, CPT, D], _F32, name=f"xt_{s}_{i}", bufs=1)
            nc.sync.dma_start(out=t[:], in_=x5[s, i])
            x_tiles[(s, i)] = t

    for s in range(SPB):
        pool_ps = psp.tile([1, D], _F32, name=f"pool_ps_{s}")
        z_ps = psp.tile([1, 1], _F32, name=f"z_ps_{s}")
        for i in range(TILES):
            xt = x_tiles[(s, i)]
            for c in range(CPT):
                col = i * CPT + c
                dump = scr.tile([128, 1], _F32, name="dump")
                nc.vector.affine_mul_reduce(
                    out=dump.broadcast_to((128, D)),
                    accum_out=s_sb[:, s, col:col + 1],
                    in0=xt[:, c, :],
                    in1=u_sb[:],
                    scale=1.0,
                    bias=mb_sb[:, s, col:col + 1] if mask_in_amr else 0.0,
                )
        if not mask_in_amr:
            nc.vector.tensor_add(s_sb[:, s, :], s_sb[:, s, :], mb_sb[:, s, :])
        # e = exp(s - C); masked rows arrive at ~MASKED_INIT -> exp == 0.
        # accum_out gives the per-partition partial sums of Z for free.
        nc.scalar.activation(e_sb[:, s, :], s_sb[:, s, :],
                             mybir.ActivationFunctionType.Exp,
                             bias=shift_sb[:] if mask_in_amr else 0.0,
                             accum_out=zc_sb[:, s:s + 1])
        for i in range(TILES):
            xt = x_tiles[(s, i)]
            for c in range(CPT):
                col = i * CPT + c
                nc.tensor.matmul(
                    pool_ps[:],
                    e_sb[:, s, col:col + 1],
                    xt[:, c, :],
                    start=(i == 0 and c == 0),
                    stop=(i == TILES - 1 and c == CPT - 1),
                )
        nc.tensor.matmul(z_ps[:], ones_sb[:], zc_sb[:, s:s + 1],
                         start=True, stop=True)
        zi_sb = sm.tile([1, 1], _F32, name=f"zi_{s}")
        nc.vector.reciprocal(zi_sb[:], z_ps[:])
        o_sb = sm.tile([1, D], _F32, name=f"o_{s}")
        nc.scalar.activation(o_sb[:], pool_ps[:],
                             mybir.ActivationFunctionType.Copy,
                             scale=zi_sb[:])
        nc.sync.dma_start(out=out[s:s + 1, :], in_=o_sb[:])


def _get_program(mask_in_amr=True):
    key = ("nc", mask_in_amr)
    if key not in _CACHE:
        _CACHE[key] = _build_program(mask_in_amr=mask_in_amr)
    return _CACHE[key]


def _prep_inputs(x, flat_mask, W, v):
    x = np.ascontiguousarray(x, dtype=np.float32)
    W = np.asarray(W, dtype=np.float32)
    v = np.asarray(v, dtype=np.float32)
    # scores = x @ u + (b . v); the constant is dropped by softmax invariance.
    u = (v @ W).astype(np.float32)
    u_rep = np.ascontiguousarray(np.broadcast_to(u, (128, D)), dtype=np.float32)

    s_u = float(u.astype(np.float64).sum())
    mask_in_amr = abs(s_u) > 1e-3
    if mask_in_amr:
        # bias slot of AFFINE_MUL_REDUCE: accum = x@u + bias*S_u
        mb = np.where(np.asarray(flat_mask) == 1,
                      np.float32(0.0), np.float32(MASKED_INIT / s_u))
    else:
        # additive fallback: s + mb before exp
        mb = np.where(np.asarray(flat_mask) == 1,
                      np.float32(-C_SHIFT), np.float32(MASKED_INIT))
    # [B, N] -> [B, 128, COLS] with [b, p, col] <- n = col*128 + p
    mb = np.ascontiguousarray(
        mb.reshape(B, COLS, 128).transpose(0, 2, 1).astype(np.float32))

    in_maps = []
    for core in range(N_CORES):
        lo = core * SPB
        in_maps.append({
            "x": np.ascontiguousarray(x[lo:lo + SPB]),
            "mb": np.ascontiguousarray(mb[lo:lo + SPB]),
            "u": u_rep,
        })
    return in_maps, mask_in_amr


def kernel(x, flat_mask, W, b, v, **_unused):
    in_maps, mask_in_amr = _prep_inputs(x, flat_mask, W, v)
    nc = _get_program(mask_in_amr)
    res = run_bass_kernel_spmd(nc, in_maps, core_ids=list(range(N_CORES)))
    return np.concatenate([res.results[i]["out"] for i in range(N_CORES)],
                          axis=0)


# revision 9
# speedup vs baseline: 36.6191x; 36.6191x over previous
"""Trainium2 Bass kernel for masked attention-pooling (DmasifAttentionModule).

Reference computation (per sample b):
    proj   = x @ W.T + b                  # [N, D]
    scores = proj @ v                     # [N]
    scores = where(mask, scores, -1e9)
    w      = softmax(scores)              # [N]
    out    = w @ x                        # [D]

Algebraic collapse used here (exact up to fp reassociation):
    scores = x @ (W.T @ v) + (b . v)
and softmax is shift-invariant, so the (b . v) constant drops out entirely.
With u = v @ W (a 512-vector computed on host), the device work is:
    s[n]  = sum_d (x[n,d] + mbias[n]) * u[d]   # = x@u + mbias[n]*S_u
    e     = exp(s - C)                         # C via a [128,1] bias tile
    Z     = sum_n e[n]                         # exp accum_out partials
    out   = (sum_n e[n] * x[n,:]) / Z

The mask rides scalar_tensor_tensor's per-partition scalar slot
(out = (in0 + mbias) * in1, accum_out = row sums): mbias[n] = 0 for valid
rows (bit-exact scores) and MASKED_INIT/S_u for masked rows (their score
lands at ~MASKED_INIT, so exp underflows to exactly 0). S_u = sum(u) is
computed on host; if it is degenerately small we fall back to an explicit
DVE mask-add before the exp.

Per-core structure (8 cores, 2 samples each, data-parallel over batch):
    - x shard [2, 4096, 512] f32 streamed as 16 x 1MiB tiles [128, 4, 512]
      (partition = n%128, free = (n//128 % 4, d)), samples interleaved in DMA
      order; tiles stay resident in SBUF so x is read from HBM exactly once.
    - per tile: 4x DVE scalar_tensor_tensor (fused mul + accum-reduce,
      ~620 ns/op), 1x ScalarE exp [128,4] (bias = -C tile, accum_out = Z
      partials), 4x TensorE pooling matmuls (lhsT = e column [128,1],
      rhs = x chunk [128,512]) accumulating into PSUM [1,512].
    - per sample: Z = ones^T @ (reduced partials), reciprocal, scaled copy
      of the PSUM accumulator, 2KB output DMA.
    (The native tensor_tensor_reduce opcode hard-crashes this runtime;
    AFFINE_MUL_REDUCE works but is ~13% slower than STT.)
This is HBM-bandwidth bound: 16 MiB/core read once (~47 us at ~358 GB/s),
with DVE busy ~41 us and the per-tile chain keeping the post-DMA tail ~5 us.
"""

import os
import sys

import numpy as np

for _p in ("/opt/trn_rl_repo", "/root/.axon_site/_ro/trn_rl_repo"):
    if os.path.isdir(_p) and _p not in sys.path:
        sys.path.append(_p)

import concourse.bacc as bacc
import concourse.tile as tile
from concourse import mybir
from concourse.bass_utils import run_bass_kernel_spmd

B, N, D = 16, 4096, 512
N_CORES = 8
SPB = B // N_CORES          # samples per core
TILES = 8                   # 1MiB x-tiles per sample
COLS = N // 128             # 32 score columns of 128 n's per sample
CPT = COLS // TILES         # score columns per tile (4)
C_SHIFT = 24.0              # constant exp-range shift (softmax-invariant)
MASKED_INIT = -3.0e8        # masked scores -> exp underflows to exactly 0

_F32 = mybir.dt.float32
_CACHE = {}


def _build_program(unroll=1, mask_in_stt=True, loop_n=None):
    """unroll/loop_n repeat the computation inside one NEFF (timing only).

    mask_in_stt=True folds the mask into the STT scalar slot
    (mb input = 0 / MASKED_INIT/S_u); False applies mb additively with a
    DVE tensor_add before the exp (mb input = -C / MASKED_INIT)."""
    nc = bacc.Bacc("TRN2", target_bir_lowering=False, debug=False)
    x = nc.dram_tensor("x", [SPB, N, D], _F32, kind="ExternalInput").ap()
    mb = nc.dram_tensor("mb", [SPB, 128, COLS], _F32, kind="ExternalInput").ap()
    u = nc.dram_tensor("u", [128, D], _F32, kind="ExternalInput").ap()
    out = nc.dram_tensor("out", [SPB, D], _F32, kind="ExternalOutput").ap()

    # [s, i, p, c, d]: n = i*512 + c*128 + p
    x5 = x.rearrange("s (i c p) d -> s i p c d", i=TILES, c=CPT, p=128)

    with tile.TileContext(nc) as tc:
        with (
            tc.tile_pool(name="xp", bufs=1) as xp,
            tc.tile_pool(name="singles", bufs=1) as sg,
            tc.tile_pool(name="scratch", bufs=4) as scr,
            tc.tile_pool(name="smalls", bufs=2) as sm,
            tc.tile_pool(name="ps", bufs=2, space="PSUM") as psp,
        ):
            ones_sb = sg.tile([128, 1], _F32)
            nc.vector.memset(ones_sb[:], 1.0)
            shift_sb = sg.tile([128, 1], _F32)
            nc.vector.memset(shift_sb[:], -C_SHIFT)
            warm = sg.tile([128, 1], _F32)
            # Pull the exp table-set load (~2.7us) to t=0, under the DMAs.
            nc.scalar.activation(warm[:], ones_sb[:],
                                 mybir.ActivationFunctionType.Exp)

            u_sb = sg.tile([128, D], _F32)
            nc.sync.dma_start(out=u_sb[:], in_=u[:])
            mb_sb = sg.tile([128, SPB, COLS], _F32)
            nc.sync.dma_start(out=mb_sb[:], in_=mb.rearrange("s p c -> p s c"))

            s_sb = sg.tile([128, SPB, COLS], _F32)
            e_sb = sg.tile([128, SPB, COLS], _F32)
            zb_sb = sg.tile([128, SPB, TILES], _F32)
            zc_sb = sg.tile([128, SPB], _F32)
            ctx = (nc, xp, scr, sm, psp, x5, out, u_sb, mb_sb, ones_sb,
                   shift_sb, s_sb, e_sb, zb_sb, zc_sb, mask_in_stt)

            if loop_n is not None:
                with tc.For_i(0, loop_n, 1) as _i:
                    _emit_iteration(*ctx)
            else:
                for _it in range(unroll):
                    _emit_iteration(*ctx)

    nc.compile()
    return nc


def _emit_iteration(nc, xp, scr, sm, psp, x5, out, u_sb, mb_sb, ones_sb,
                    shift_sb, s_sb, e_sb, zb_sb, zc_sb, mask_in_stt):
    # DMA all 16 tiles up front, samples interleaved, so DVE/ACT/PE chase
    # the DMA stream tile by tile.
    order = [(s, i) for i in range(TILES) for s in range(SPB)]
    x_tiles = {}
    for s, i in order:
        t = xp.tile([128, CPT, D], _F32, name=f"xt_{s}_{i}", bufs=1)
        nc.sync.dma_start(out=t[:], in_=x5[s, i])
        x_tiles[(s, i)] = t

    pool_ps = {}
    for s in range(SPB):
        pool_ps[s] = psp.tile([1, D], _F32, name=f"pool_ps_{s}")

    for s, i in order:
        xt = x_tiles[(s, i)]
        c0 = i * CPT
        for c in range(CPT):
            col = c0 + c
            dump = scr.tile([128, 1], _F32, name="dump")
            nc.vector.scalar_tensor_tensor(
                out=dump.broadcast_to((128, D)),
                in0=xt[:, c, :],
                scalar=mb_sb[:, s, col:col + 1] if mask_in_stt else 0.0,
                in1=u_sb[:],
                op0=mybir.AluOpType.add,
                op1=mybir.AluOpType.mult,
                accum_out=s_sb[:, s, col:col + 1],
            )
        if not mask_in_stt:
            nc.vector.tensor_add(s_sb[:, s, c0:c0 + CPT],
                                 s_sb[:, s, c0:c0 + CPT],
                                 mb_sb[:, s, c0:c0 + CPT])
        # e = exp(s - C); masked rows arrive at ~MASKED_INIT -> exp == 0.
        # accum_out collects this tile's per-partition partial Z sums.
        nc.scalar.activation(e_sb[:, s, c0:c0 + CPT], s_sb[:, s, c0:c0 + CPT],
                             mybir.ActivationFunctionType.Exp,
                             bias=shift_sb[:] if mask_in_stt else 0.0,
                             accum_out=zb_sb[:, s, i:i + 1])
        for c in range(CPT):
            col = c0 + c
            nc.tensor.matmul(
                pool_ps[s][:],
                e_sb[:, s, col:col + 1],
                xt[:, c, :],
                start=(i == 0 and c == 0),
                stop=(i == TILES - 1 and c == CPT - 1),
            )

    for s in range(SPB):
        z_ps = psp.tile([1, 1], _F32, name=f"z_ps_{s}")
        nc.vector.tensor_reduce(zc_sb[:, s:s + 1], zb_sb[:, s, :],
                                axis=mybir.AxisListType.X,
                                op=mybir.AluOpType.add)
        nc.tensor.matmul(z_ps[:], ones_sb[:], zc_sb[:, s:s + 1],
                         start=True, stop=True)
        zi_sb = sm.tile([1, 1], _F32, name=f"zi_{s}")
        nc.vector.reciprocal(zi_sb[:], z_ps[:])
        o_sb = sm.tile([1, D], _F32, name=f"o_{s}")
        nc.scalar.activation(o_sb[:], pool_ps[s][:],
                             mybir.ActivationFunctionType.Copy,
                             scale=zi_sb[:])
        nc.sync.dma_start(out=out[s:s + 1, :], in_=o_sb[:])


def _get_program(mask_in_stt=True):
    key = ("nc", mask_in_stt)
    if key not in _CACHE:
        _CACHE[key] = _build_program(mask_in_stt=mask_in_stt)
    return _CACHE[key]


def _prep_inputs(x, flat_mask, W, v):
    x = np.ascontiguousarray(x, dtype=np.float32)
    W = np.asarray(W, dtype=np.float32)
    v = np.asarray(v, dtype=np.float32)
    # scores = x @ u + (b . v); the constant is dropped by softmax invariance.
    u = (v @ W).astype(np.float32)
    u_rep = np.ascontiguousarray(np.broadcast_to(u, (128, D)), dtype=np.float32)

    s_u = float(u.astype(np.float64).sum())
    mask_in_stt = abs(s_u) > 1e-3
    if mask_in_stt:
        # scalar slot of STT: accum = x@u + mbias*S_u
        mb = np.where(np.asarray(flat_mask) == 1,
                      np.float32(0.0), np.float32(MASKED_INIT / s_u))
    else:
        # additive fallback: s + mb before exp
        mb = np.where(np.asarray(flat_mask) == 1,
                      np.float32(-C_SHIFT), np.float32(MASKED_INIT))
    # [B, N] -> [B, 128, COLS] with [b, p, col] <- n = col*128 + p
    mb = np.ascontiguousarray(
        mb.reshape(B, COLS, 128).transpose(0, 2, 1).astype(np.float32))

    in_maps = []
    for core in range(N_CORES):
        lo = core * SPB
        in_maps.append({
            "x": np.ascontiguousarray(x[lo:lo + SPB]),
            "mb": np.ascontiguousarray(mb[lo:lo + SPB]),
            "u": u_rep,
        })
    return in_maps, mask_in_stt


def kernel(x, flat_mask, W, b, v, **_unused):
    in_maps, mask_in_stt = _prep_inputs(x, flat_mask, W, v)
    nc = _get_program(mask_in_stt)
    res = run_bass_kernel_spmd(nc, in_maps, core_ids=list(range(N_CORES)))
    return np.concatenate([res.results[i]["out"] for i in range(N_CORES)],
                          axis=0)


# revision 11
# speedup vs baseline: 56.8187x; 1.5516x over previous
"""Trainium2 Bass kernel for masked attention-pooling (DmasifAttentionModule).

Reference computation (per sample b):
    proj   = x @ W.T + b                  # [N, D]
    scores = proj @ v                     # [N]
    scores = where(mask, scores, -1e9)
    w      = softmax(scores)              # [N]
    out    = w @ x                        # [D]

Optimizations (all exact up to fp reassociation):
  1. scores = x @ (W.T @ v) + (b . v); softmax is shift-invariant, so the
     (b . v) constant drops out and the 34-GFLOP projection collapses to a
     matvec against u = v @ W (host-computed, 512 floats).
  2. Masked rows get softmax weight exactly 0, so only the ~50% valid rows
     participate at all. The host compacts each sample to its valid rows
     (padded to a common column count with zero rows + masked bias), and the
     device streams only the compacted tensor.
  3. Device per sample (nc = valid columns of 128 rows):
         s[q]  = sum_d (x[q,d] + mbias[q]) * u[d]    # = x@u (mbias=0 valid,
                                                     #   MASKED/S_u padding)
         e     = exp(s - C)                          # C via [128,1] bias tile
         Z     = sum e                               # exp accum_out partials
         out   = (sum_q e[q] * x[q,:]) / Z

Per-core structure (8 cores, 2 samples each, data-parallel over batch):
    - compacted x shard [2, NCAP, D] f32 streamed as <=1MiB tiles
      [128, <=4, 512] (partition = row%128), samples interleaved in DMA
      order; tiles stay resident in SBUF (read from HBM exactly once).
    - scores: DVE scalar_tensor_tensor (fused (x+mb)*u with accum-reduce,
      ~620 ns per [128,512]; the native tensor_tensor_reduce opcode
      hard-crashes this runtime and AFFINE_MUL_REDUCE is ~13% slower).
    - exp + Z partials: ScalarE activation per tile, bias = -C tile,
      accum_out = per-partition partial sums of e.
    - pooling + Z: TensorE matvec accumulation into PSUM [1,512]
      (lhsT = e column [128,1], rhs = x chunk [128,512]; fp32 rhs streams at
      ~871 ns per 512-col chunk, which is the PE fp32 floor).
    - finalize per sample: Z = ones^T @ partials, DVE reciprocal, ScalarE
      scaled copy of the PSUM accumulator, 2KB output DMA.
"""

import os
import sys

import numpy as np

for _p in ("/opt/trn_rl_repo", "/root/.axon_site/_ro/trn_rl_repo"):
    if os.path.isdir(_p) and _p not in sys.path:
        sys.path.append(_p)

import concourse.bacc as bacc
import concourse.tile as tile
from concourse import mybir
from concourse.bass_utils import run_bass_kernel_spmd

B, N, D = 16, 4096, 512
N_CORES = 8
SPB = B // N_CORES          # samples per core
CPT = 4                     # score columns (of 128 rows) per x tile
C_SHIFT = 24.0              # constant exp-range shift (softmax-invariant)
MASKED_INIT = -3.0e8        # masked scores -> exp underflows to exactly 0

_F32 = mybir.dt.float32
_CACHE = {}


def _build_program(ncols, mask_in_stt=True, loop_n=None):
    """Program for samples compacted to `ncols` columns of 128 rows each.

    loop_n wraps the computation in a HW For_i loop (timing only).
    mask_in_stt=True folds the mask into the STT scalar slot
    (mb input = 0 / MASKED_INIT/S_u); False applies mb additively with a
    DVE tensor_add before the exp (mb input = -C / MASKED_INIT)."""
    ncap = ncols * 128
    tiles = [(c0, min(CPT, ncols - c0)) for c0 in range(0, ncols, CPT)]

    nc = bacc.Bacc("TRN2", target_bir_lowering=False, debug=False)
    x = nc.dram_tensor("x", [SPB, ncap, D], _F32, kind="ExternalInput").ap()
    mb = nc.dram_tensor("mb", [SPB, 128, ncols], _F32,
                        kind="ExternalInput").ap()
    u = nc.dram_tensor("u", [128, D], _F32, kind="ExternalInput").ap()
    out = nc.dram_tensor("out", [SPB, D], _F32, kind="ExternalOutput").ap()

    # [s, p, q, d]: row = q*128 + p
    x4 = x.rearrange("s (q p) d -> s p q d", p=128)

    with tile.TileContext(nc) as tc:
        with (
            tc.tile_pool(name="xp", bufs=1) as xp,
            tc.tile_pool(name="singles", bufs=1) as sg,
            tc.tile_pool(name="scratch", bufs=4) as scr,
            tc.tile_pool(name="smalls", bufs=2) as sm,
            tc.tile_pool(name="ps", bufs=2, space="PSUM") as psp,
        ):
            ones_sb = sg.tile([128, 1], _F32)
            nc.vector.memset(ones_sb[:], 1.0)
            shift_sb = sg.tile([128, 1], _F32)
            nc.vector.memset(shift_sb[:], -C_SHIFT)
            warm = sg.tile([128, 1], _F32)
            # Pull the exp table-set load (~2.7us) to t=0, under the DMAs.
            nc.scalar.activation(warm[:], ones_sb[:],
                                 mybir.ActivationFunctionType.Exp)

            u_sb = sg.tile([128, D], _F32)
            nc.sync.dma_start(out=u_sb[:], in_=u[:])
            mb_sb = sg.tile([128, SPB, ncols], _F32)
            nc.sync.dma_start(out=mb_sb[:], in_=mb.rearrange("s p c -> p s c"))

            s_sb = sg.tile([128, SPB, ncols], _F32)
            e_sb = sg.tile([128, SPB, ncols], _F32)
            zb_sb = sg.tile([128, SPB, len(tiles)], _F32)
            zc_sb = sg.tile([128, SPB], _F32)
            ctx = (nc, xp, scr, sm, psp, x4, out, u_sb, mb_sb, ones_sb,
                   shift_sb, s_sb, e_sb, zb_sb, zc_sb, tiles, mask_in_stt)

            if loop_n is not None:
                with tc.For_i(0, loop_n, 1) as _i:
                    _emit_iteration(*ctx)
            else:
                _emit_iteration(*ctx)

    nc.compile()
    return nc


def _emit_iteration(nc, xp, scr, sm, psp, x4, out, u_sb, mb_sb, ones_sb,
                    shift_sb, s_sb, e_sb, zb_sb, zc_sb, tiles, mask_in_stt):
    # DMA all tiles up front, samples interleaved, so DVE/ACT/PE chase the
    # DMA stream tile by tile.
    order = [(s, ti) for ti in range(len(tiles)) for s in range(SPB)]
    x_tiles = {}
    for s, ti in order:
        c0, cw = tiles[ti]
        t = xp.tile([128, cw, D], _F32, name=f"xt_{s}_{ti}", bufs=1)
        nc.sync.dma_start(out=t[:], in_=x4[s, :, c0:c0 + cw, :])
        x_tiles[(s, ti)] = t

    pool_ps = {}
    for s in range(SPB):
        pool_ps[s] = psp.tile([1, D], _F32, name=f"pool_ps_{s}")

    last = order[-1]
    for s, ti in order:
        xt = x_tiles[(s, ti)]
        c0, cw = tiles[ti]
        for c in range(cw):
            col = c0 + c
            dump = scr.tile([128, 1], _F32, name="dump")
            nc.vector.scalar_tensor_tensor(
                out=dump.broadcast_to((128, D)),
                in0=xt[:, c, :],
                scalar=mb_sb[:, s, col:col + 1] if mask_in_stt else 0.0,
                in1=u_sb[:],
                op0=mybir.AluOpType.add,
                op1=mybir.AluOpType.mult,
                accum_out=s_sb[:, s, col:col + 1],
            )
        if not mask_in_stt:
            nc.vector.tensor_add(s_sb[:, s, c0:c0 + cw],
                                 s_sb[:, s, c0:c0 + cw],
                                 mb_sb[:, s, c0:c0 + cw])
        # e = exp(s - C); padding rows arrive at ~MASKED_INIT -> exp == 0.
        # accum_out collects this tile's per-partition partial Z sums.
        nc.scalar.activation(e_sb[:, s, c0:c0 + cw], s_sb[:, s, c0:c0 + cw],
                             mybir.ActivationFunctionType.Exp,
                             bias=shift_sb[:] if mask_in_stt else 0.0,
                             accum_out=zb_sb[:, s, ti:ti + 1])
        for c in range(cw):
            col = c0 + c
            nc.tensor.matmul(
                pool_ps[s][:],
                e_sb[:, s, col:col + 1],
                xt[:, c, :],
                start=(ti == 0 and c == 0),
                stop=(ti == len(tiles) - 1 and c == cw - 1),
            )

    for s in range(SPB):
        z_ps = psp.tile([1, 1], _F32, name=f"z_ps_{s}")
        nc.vector.tensor_reduce(zc_sb[:, s:s + 1], zb_sb[:, s, :],
                                axis=mybir.AxisListType.X,
                                op=mybir.AluOpType.add)
        nc.tensor.matmul(z_ps[:], ones_sb[:], zc_sb[:, s:s + 1],
                         start=True, stop=True)
        zi_sb = sm.tile([1, 1], _F32, name=f"zi_{s}")
        nc.vector.reciprocal(zi_sb[:], z_ps[:])
        o_sb = sm.tile([1, D], _F32, name=f"o_{s}")
        nc.scalar.activation(o_sb[:], pool_ps[s][:],
                             mybir.ActivationFunctionType.Copy,
                             scale=zi_sb[:])
        nc.sync.dma_start(out=out[s:s + 1, :], in_=o_sb[:])


def _get_program(ncols, mask_in_stt=True):
    key = (ncols, mask_in_stt)
    if key not in _CACHE:
        _CACHE[key] = _build_program(ncols, mask_in_stt=mask_in_stt)
    return _CACHE[key]


def _prep_inputs(x, flat_mask, W, v):
    """Compact to valid rows; returns (in_maps, meta)."""
    x = np.ascontiguousarray(x, dtype=np.float32)
    flat_mask = np.asarray(flat_mask)
    W = np.asarray(W, dtype=np.float32)
    v = np.asarray(v, dtype=np.float32)
    # scores = x @ u + (b . v); the constant is dropped by softmax invariance.
    u = (v @ W).astype(np.float32)
    u_rep = np.ascontiguousarray(np.broadcast_to(u, (128, D)), dtype=np.float32)

    s_u = float(u.astype(np.float64).sum())
    mask_in_stt = abs(s_u) > 1e-3
    masked_val = np.float32(MASKED_INIT / s_u) if mask_in_stt \
        else np.float32(MASKED_INIT)
    valid_val = np.float32(0.0) if mask_in_stt else np.float32(-C_SHIFT)

    idxs = [np.nonzero(flat_mask[b] == 1)[0] for b in range(B)]
    counts = np.array([len(ix) for ix in idxs])
    ncols = max(1, int(-(-counts.max() // 128)))
    ncap = ncols * 128

    xc = np.zeros((B, ncap, D), dtype=np.float32)
    mbc = np.full((B, ncap), masked_val, dtype=np.float32)
    for b in range(B):
        cnt = counts[b]
        if cnt:
            xc[b, :cnt] = x[b, idxs[b]]
            mbc[b, :cnt] = valid_val
    # [B, ncap] -> [B, 128, ncols] with [b, p, col] <- row = col*128 + p
    mbc = np.ascontiguousarray(
        mbc.reshape(B, ncols, 128).transpose(0, 2, 1))

    in_maps = []
    for core in range(N_CORES):
        lo = core * SPB
        in_maps.append({
            "x": np.ascontiguousarray(xc[lo:lo + SPB]),
            "mb": np.ascontiguousarray(mbc[lo:lo + SPB]),
            "u": u_rep,
        })
    meta = {"ncols": ncols, "mask_in_stt": mask_in_stt, "counts": counts}
    return in_maps, meta


def kernel(x, flat_mask, W, b, v, **_unused):
    in_maps, meta = _prep_inputs(x, flat_mask, W, v)
    nc = _get_program(meta["ncols"], meta["mask_in_stt"])
    res = run_bass_kernel_spmd(nc, in_maps, core_ids=list(range(N_CORES)))
    out = np.concatenate([res.results[i]["out"] for i in range(N_CORES)],
                         axis=0)
    if (meta["counts"] == 0).any():
        # Reference semantics for an all-masked sample: uniform mean pool.
        x = np.asarray(x, dtype=np.float32)
        for bi in np.nonzero(meta["counts"] == 0)[0]:
            out[bi] = x[bi].mean(axis=0)
    return out
